# revision 4
# baseline (speedup 1.0000x reference)
"""Trainium2 Bass kernel for a 2-layer cross-encoder (CrossEncoder).

Model: B=2, NQ=NKV=2048, E=512, H=8 (d_head=64), MLP=2048, depth=2, fp32 I/O.

Sharding (8 cores, no collectives): core c handles batch b=c//4 and query
rows [qc*512, (qc+1)*512) with qc=c%4.  Each core computes the full KV
projections for its batch so every core produces its output slice
independently.

Numerics: heavy matmuls (Q/K/V/O projections, FFN, attn@V) run in fp8e4m3
with DoubleRow perf mode (2 k-chunks contracted per pass).  Weights are
pre-scaled x64 on the host so their mass sits in fp8's normal range; the
inverse scale is folded into the PSUM consumers.  q/k are stored x8 in
bf16 (scores matmul is fp8-rate anyway, bf16 costs the same and is more
accurate).  exp() of the scores is split between the Activation engine
(true Exp, fp8 out) and DVE (Schraudolph bit-trick exp directly into fp8
bits via an int8 round).  The softmax denominator comes free from 64
constant columns appended per head to V (rows 64..127 of the attn@V psum
all hold the per-query sum), so the normalizer is one reciprocal + one
multiply, no replicate matmul.  LayerNorm statistics, softmax
normalization and the residual stream stay fp32.  LN affine params and
all biases are folded into weights / matmul bias-rows on the host; the
k-bias is dropped entirely (softmax is invariant to it).
"""

import numpy as np
import ml_dtypes

import concourse.bass as bass
import concourse.bacc as bacc
import concourse.mybir as mybir
import concourse.tile as tile
from concourse import bass_utils, masks
from contextlib import ExitStack

P = 128
E = 512
EC = E // P        # 4 chunks of the embedding dim
SE = EC // 2       # 2 DoubleRow super-chunks
NQ = 512           # query rows per core
QC = NQ // P       # 4 query chunks
NKV = 2048
KC = NKV // P      # 16 key chunks of 128
KN = NKV // 512    # 4 key chunks of 512
GK = KC // 2       # 8 key pair-groups
H = 8
DH = 64
MLP = 2048
MC = MLP // P      # 16 mlp chunks of 128
SM = MC // 2       # 8 DoubleRow super-chunks
L = 2
LN_EPS = 1e-5
F32 = mybir.dt.float32
BF16 = mybir.dt.bfloat16
FP8 = mybir.dt.float8e4
I8 = mybir.dt.int8
AF = mybir.ActivationFunctionType
ALU = mybir.AluOpType
DRM = mybir.MatmulPerfMode.DoubleRow

WS = 64.0                       # fp8 weight pre-scale (host side)
QKS = 8.0                       # q/k storage scale
SCALE = DH ** -0.5
EXPS = SCALE / (QKS * QKS)      # exp scale applied to scores psum (=1/512)
AOS = 64.0                      # attnout storage scale (fp8 subnormal guard)
SCH_A = (8.0 / np.log(2.0)) * EXPS   # Schraudolph slope for fp8e4 bits
SCH_B = 56.0 - 0.47                  # fp8e4 exponent bias term - rms shift
ACT_EXP = 5                     # of the 8 exp tiles per head, Act takes [0..5)

_CACHE = {}


def _build():
    """Build the per-core Bass program (identical on all 8 cores)."""
    nc = bacc.Bacc("TRN2", target_bir_lowering=False, debug=False, num_devices=8)

    xq_d = nc.dram_tensor("xq", [NQ, E], F32, kind="ExternalInput").ap()
    xkv_d = nc.dram_tensor("xkv", [NKV, E], F32, kind="ExternalInput").ap()
    wd = []
    for l in range(L):
        wd.append({
            "wq8": nc.dram_tensor(f"wq8_{l}", [P, SE * 2 * E], FP8, kind="ExternalInput").ap(),
            "wk8": nc.dram_tensor(f"wk8_{l}", [P, SE * 2 * E], FP8, kind="ExternalInput").ap(),
            "wv8": nc.dram_tensor(f"wv8_{l}", [P, SE * 2 * E], FP8, kind="ExternalInput").ap(),
            "wo8": nc.dram_tensor(f"wo8_{l}", [P, SE * 2 * E], FP8, kind="ExternalInput").ap(),
            "w18": nc.dram_tensor(f"w18_{l}", [P, SE * 2 * MLP], FP8, kind="ExternalInput").ap(),
            "w28": nc.dram_tensor(f"w28_{l}", [P, SM * 2 * E], FP8, kind="ExternalInput").ap(),
            "bq": nc.dram_tensor(f"bq_{l}", [P, EC], F32, kind="ExternalInput").ap(),
            "b1": nc.dram_tensor(f"b1_{l}", [P, MC], F32, kind="ExternalInput").ap(),
            "bo_row": nc.dram_tensor(f"bo_row_{l}", [1, E], BF16, kind="ExternalInput").ap(),
            "b2_row": nc.dram_tensor(f"b2_row_{l}", [1, E], BF16, kind="ExternalInput").ap(),
        })
    y_d = nc.dram_tensor("y", [NQ, E], F32, kind="ExternalOutput").ap()

    with tile.TileContext(nc) as tc, ExitStack() as ctx:
        const_pool = ctx.enter_context(tc.tile_pool(name="const", bufs=1))
        ident = const_pool.tile([P, P], BF16)
        masks.make_identity(nc, ident)
        ones1 = const_pool.tile([1, P], BF16)
        nc.gpsimd.memset(ones1[:], 1.0)
        eps_col = const_pool.tile([P, 1], F32)
        nc.gpsimd.memset(eps_col[:], LN_EPS)

        stats_pool = ctx.enter_context(tc.tile_pool(name="stats", bufs=12))

        def ln_rstd(x_ap):
            """LayerNorm stats: returns (bnag, rstd) [P,1] tiles (fp32)."""
            bnst = stats_pool.tile([P, 6], F32, name="bnst")
            nc.vector.bn_stats(bnst[:], x_ap)
            bnag = stats_pool.tile([P, 2], F32, name="bnag")
            nc.vector.bn_aggr(bnag[:], bnst[:])
            sq = stats_pool.tile([P, 1], F32, name="sq")
            nc.scalar.activation(sq[:], bnag[:, 1:2], AF.Sqrt, bias=eps_col[:])
            rstd = stats_pool.tile([P, 1], F32, name="rstd")
            nc.vector.reciprocal(rstd[:], sq[:])
            return bnag, rstd

        # Residual stream: 4 fp32 tiles of [128, 512].
        xq_pool = ctx.enter_context(tc.tile_pool(name="xq", bufs=1))
        xq = []
        for i in range(QC):
            t = xq_pool.tile([P, E], F32, name=f"xq{i}", tag=f"xq{i}")
            nc.sync.dma_start(t[:], xq_d[i * P:(i + 1) * P, :])
            xq.append(t[:])

        # hkv^T in fp8 DoubleRow pair layout: tile s holds E-chunks (2s, 2s+1)
        # as the pair dim -> [128, 2, NKV].
        hkvT_pool = ctx.enter_context(tc.tile_pool(name="hkvT", bufs=1))
        hkvT = [
            hkvT_pool.tile([P, 2, NKV], FP8, name=f"hkvT{s}", tag=f"hkvT{s}")
            for s in range(SE)
        ]

        # PSUM pools (8 banks): pp 2 + ss 2x2 + att 2 = 8.
        pp_pool = ctx.enter_context(tc.tile_pool(name="pp", bufs=2, space="PSUM"))
        ss_pool = ctx.enter_context(tc.tile_pool(name="ss", bufs=2, space="PSUM"))
        att_pool = ctx.enter_context(tc.tile_pool(name="attp", bufs=2, space="PSUM"))

        # Weight tiles for both layers live in SBUF simultaneously.
        wpool = ctx.enter_context(tc.tile_pool(name="w", bufs=1))

        def alloc_w_crit(l):
            d = {}
            for nm, sz in (("wq8", SE * 2 * E), ("wk8", SE * 2 * E), ("wv8", SE * 2 * E)):
                d[nm] = wpool.tile([P, sz], FP8, name=f"{nm}_{l}")
                nc.sync.dma_start(d[nm][:], wd[l][nm])
            d["bq"] = wpool.tile([P, EC], F32, name=f"bq_{l}")
            nc.sync.dma_start(d["bq"][:], wd[l]["bq"])
            return d

        def alloc_w_rest(d, l):
            for nm, sz, dt in (("wo8", SE * 2 * E, FP8), ("w18", SE * 2 * MLP, FP8),
                               ("w28", SM * 2 * E, FP8)):
                d[nm] = wpool.tile([P, sz], dt, name=f"{nm}_{l}")
                nc.sync.dma_start(d[nm][:], wd[l][nm])
            d["b1"] = wpool.tile([P, MC], F32, name=f"b1_{l}")
            nc.sync.dma_start(d["b1"][:], wd[l]["b1"])
            d["bo_row"] = wpool.tile([1, E], BF16, name=f"bo_row_{l}")
            nc.sync.dma_start(d["bo_row"][:], wd[l]["bo_row"])
            d["b2_row"] = wpool.tile([1, E], BF16, name=f"b2_row_{l}")
            nc.sync.dma_start(d["b2_row"][:], wd[l]["b2_row"])
            return d

        def w_slice(wtile, s, c0, c1, S=SE):
            """[P, S*2*cols] fp8 tile -> [128, 2, c1-c0] DR stationary slice."""
            return wtile[:].rearrange("p (s j c) -> p s j c", s=S, j=2)[:, s, :, c0:c1]

        # LN + transpose into fp8 pair tiles.  Blocks are batched 8 per psum
        # bank (2 source tiles x 4 E-chunks) and copied out 2 blocks at a
        # time per destination pair-tile.
        def ln_transpose_pair(src0, src1, dstT, tok0, copy_par, apply_eng0, apply_eng1):
            pt = pp_pool.tile([P, E], F32, name="pp", tag="pp")
            ptb = pt[:].bitcast(BF16)
            for sub, (src, eng) in enumerate(((src0, apply_eng0), (src1, apply_eng1))):
                bnag, rstd = ln_rstd(src)
                hq_t = work.tile([P, E], BF16, name="hq_t", bufs=4)
                eng.tensor_scalar(
                    hq_t[:], src, bnag[:, 0:1], rstd[:], op0=ALU.subtract, op1=ALU.mult
                )
                for e in range(EC):
                    col = (e * 2 + sub) * P
                    nc.tensor.matmul(
                        ptb[:, col:col + P], hq_t[:, e * P:(e + 1) * P], ident[:],
                        is_transpose=True,
                        start=(sub == 0 and e == 0), stop=(sub == 1 and e == EC - 1),
                        skip_group_check=True,
                    )
            for s in range(SE):
                in_ap = ptb[:, s * 512:(s + 1) * 512].rearrange(
                    "p (j u c) -> p j u c", j=2, u=2
                )
                out_ap = dstT[s][:, :, tok0:tok0 + 256].rearrange(
                    "p j (u c) -> p j u c", u=2
                )
                if (s + copy_par) % 2:
                    nc.scalar.copy(out_ap, in_ap)
                else:
                    nc.vector.tensor_copy(out_ap, in_ap)

        # ---- hkv^T setup: LN1-core of x_kv (layer-independent, g/b folded)
        w0 = None
        with tc.tile_pool(name="xkv", bufs=4) as xkv_pool:
            for g4 in range(KC // 2):
                if g4 == 2:
                    w0 = alloc_w_crit(0)
                if g4 == 5:
                    w0 = alloc_w_rest(w0, 0)
                xkv_t = xkv_pool.tile([P, 2, E], F32, name="xkv_t", tag="xkv_t")
                nc.sync.dma_start(
                    xkv_t[:],
                    xkv_d[g4 * 2 * P:(g4 + 1) * 2 * P, :].rearrange(
                        "(i p) c -> p i c", p=P
                    ),
                )
                pt = pp_pool.tile([P, E], F32, name="pp", tag="pp")
                ptb = pt[:].bitcast(BF16)
                for sub in range(2):
                    bnag, rstd = ln_rstd(xkv_t[:, sub, :])
                    hkv_t = xkv_pool.tile([P, E], BF16, name="hkv_t", tag="hkv_t")
                    (nc.gpsimd if sub else nc.vector).tensor_scalar(
                        hkv_t[:], xkv_t[:, sub, :], bnag[:, 0:1], rstd[:],
                        op0=ALU.subtract, op1=ALU.mult,
                    )
                    for e in range(EC):
                        col = (e * 2 + sub) * P
                        nc.tensor.matmul(
                            ptb[:, col:col + P], hkv_t[:, e * P:(e + 1) * P], ident[:],
                            is_transpose=True,
                            start=(sub == 0 and e == 0), stop=(sub == 1 and e == EC - 1),
                            skip_group_check=True,
                        )
                for s in range(SE):
                    in_ap = ptb[:, s * 512:(s + 1) * 512].rearrange(
                        "p (j u c) -> p j u c", j=2, u=2
                    )
                    out_ap = hkvT[s][:, :, g4 * 256:(g4 + 1) * 256].rearrange(
                        "p j (u c) -> p j u c", u=2
                    )
                    if (s + g4) % 2:
                        nc.scalar.copy(out_ap, in_ap)
                    else:
                        nc.vector.tensor_copy(out_ap, in_ap)

        # Work pools.
        work = ctx.enter_context(tc.tile_pool(name="work", bufs=1))
        big = ctx.enter_context(tc.tile_pool(name="big", bufs=1))
        ex_pool = ctx.enter_context(tc.tile_pool(name="ex", bufs=5))

        w_t = [w0, None]

        for l in range(L):
            wt = w_t[l]

            # ---- LN1(x_q) -> hqT fp8 pair tiles [128, 2, NQ] ----
            hqT = [
                work.tile([P, 2, NQ], FP8, name=f"hqT{s}", tag=f"actT{s}")
                for s in range(SE)
            ]
            for qp in range(QC // 2):
                ln_transpose_pair(
                    xq[2 * qp], xq[2 * qp + 1], hqT, qp * 256, qp,
                    nc.vector, nc.gpsimd,
                )

            # ---- q^T = wq^T @ hq^T: DR fp8, out scaled x8 + bias ----
            qT = [
                work.tile([P, NQ], BF16, name=f"qT{m}", tag=f"qT{m}")
                for m in range(EC)
            ]
            for m in range(EC):
                ps = pp_pool.tile([P, E], F32, name="pp", tag="pp")
                for s in range(SE):
                    nc.tensor.matmul(
                        ps[:], w_slice(wt["wq8"], s, m * P, (m + 1) * P),
                        hqT[s][:], start=(s == 0), stop=(s == SE - 1), perf_mode=DRM,
                    )
                nc.vector.tensor_scalar(
                    qT[m][:], ps[:], 1.0 / QKS, wt["bq"][:, m:m + 1],
                    op0=ALU.mult, op1=ALU.add,
                )

            # ---- k^T: DR fp8, out scaled x8, bias dropped ----
            kT = [
                big.tile([P, NKV], BF16, name=f"kT{m}", tag=f"kT{m}", bufs=2)
                for m in range(EC)
            ]
            for m in range(EC):
                for n in range(KN):
                    ps = pp_pool.tile([P, E], F32, name="pp", tag="pp")
                    for s in range(SE):
                        nc.tensor.matmul(
                            ps[:], w_slice(wt["wk8"], s, m * P, (m + 1) * P),
                            hkvT[s][:, :, n * 512:(n + 1) * 512],
                            start=(s == 0), stop=(s == SE - 1), perf_mode=DRM,
                        )
                    if (m + n) % 2:
                        nc.scalar.mul(kT[m][:, n * 512:(n + 1) * 512], ps[:], 1.0 / QKS)
                    else:
                        nc.vector.tensor_scalar_mul(
                            kT[m][:, n * 512:(n + 1) * 512], ps[:], 1.0 / QKS
                        )

            # ---- v: DR fp8 -> va pair tiles [128, 2, H, 128] (v x8 in cols
            #      0..63, constant 1/8 in cols 64..127 for the denominator) ----
            va = [
                big.tile([P, 2, H, 2 * DH], FP8, name=f"va{g}", tag=f"va{g}", bufs=2)
                for g in range(GK)
            ]
            for g in range(GK):
                nc.gpsimd.memset(va[g][:, :, :, DH:2 * DH], 1.0 / QKS)
            for m in range(KC):
                ps = pp_pool.tile([P, E], F32, name="pp", tag="pp")
                for s in range(SE):
                    nc.tensor.matmul(
                        ps[:], hkvT[s][:, :, m * P:(m + 1) * P],
                        w_slice(wt["wv8"], s, 0, E),
                        start=(s == 0), stop=(s == SE - 1), perf_mode=DRM,
                    )
                dst = va[m // 2][:, m % 2, :, 0:DH]
                src = ps[:].rearrange("p (h d) -> p h d", h=H)
                if m % 2:
                    nc.scalar.mul(dst, src, 1.0 / QKS)
                else:
                    nc.vector.tensor_scalar_mul(dst, src, 1.0 / QKS)

            if l == 0:
                w1c = alloc_w_crit(1)
                w_t[1] = alloc_w_rest(w1c, 1)

            # ---- attention, head by head ----
            aoT = [
                work.tile([P, 2, NQ], FP8, name=f"aoT{s}", tag=f"aoT{s}")
                for s in range(SE)
            ]
            for h in range(H):
                fh, r0 = h // 2, (h % 2) * DH
                s_ao, j_ao = fh // 2, fh % 2
                # attn@V accumulator: rows 0..63 = unnormalized attnout x8
                # (transposed), rows 64..127 = sum(ex)/8 per query.
                ps_oT = att_pool.tile([P, E], F32, name="ps_oT", tag="att")
                exs = []
                for g in range(GK):
                    ps_s = ss_pool.tile([P, 2, NQ], F32, name="ps_s", tag="ss")
                    for sub in range(2):
                        m = 2 * g + sub
                        nc.tensor.matmul(
                            ps_s[:, sub, :],
                            kT[fh][r0:r0 + DH, m * P:(m + 1) * P],
                            qT[fh][r0:r0 + DH, :],
                            start=True, stop=True,
                        )
                    ex = ex_pool.tile([P, 2, NQ], FP8, name="ex", tag="ex")
                    if g < ACT_EXP:
                        nc.scalar.activation(ex[:], ps_s[:], AF.Exp, scale=EXPS)
                    else:
                        nc.vector.tensor_scalar(
                            ex[:].bitcast(I8), ps_s[:], SCH_A, SCH_B,
                            op0=ALU.mult, op1=ALU.add,
                        )
                    exs.append(ex)
                    if g >= 1:
                        nc.tensor.matmul(
                            ps_oT[:], va[g - 1][:, :, h, :], exs[g - 1][:],
                            start=(g == 1), stop=False, perf_mode=DRM,
                        )
                nc.tensor.matmul(
                    ps_oT[:], va[GK - 1][:, :, h, :], exs[GK - 1][:],
                    start=False, stop=True, perf_mode=DRM,
                )
                # normalize: aoT rows = (x8 unnorm) * rcp(sum/8) = x64 attnout
                rcp = stats_pool.tile([DH, NQ], BF16, name="rcp", bufs=2)
                with nc.allow_low_precision(reason="bf16 softmax denominator"):
                    nc.vector.reciprocal(rcp[:], ps_oT[DH:P, :])
                nc.vector.tensor_tensor(
                    aoT[s_ao][r0:r0 + DH, j_ao, :], ps_oT[0:DH, :], rcp[:],
                    op=ALU.mult,
                )

            # ---- out-proj (DR fp8) + bo row + residual ----
            for qc in range(QC):
                ps = pp_pool.tile([P, E], F32, name="pp", tag="pp")
                for s in range(SE):
                    nc.tensor.matmul(
                        ps[:], aoT[s][:, :, qc * P:(qc + 1) * P],
                        w_slice(wt["wo8"], s, 0, E),
                        start=(s == 0), stop=False, perf_mode=DRM,
                    )
                nc.tensor.matmul(
                    ps[:], ones1[:], wt["bo_row"][:], start=False, stop=True,
                    skip_group_check=True,
                )
                nc.vector.scalar_tensor_tensor(
                    xq[qc], ps[:], 1.0 / (AOS * WS), xq[qc],
                    op0=ALU.mult, op1=ALU.add,
                )

            # ---- LN2 -> h2T ----
            h2T = [
                work.tile([P, 2, NQ], FP8, name=f"h2T{s}", tag=f"actT{s}")
                for s in range(SE)
            ]
            for qp in range(QC // 2):
                ln_transpose_pair(
                    xq[2 * qp], xq[2 * qp + 1], h2T, qp * 256, qp + 1,
                    nc.gpsimd, nc.vector,
                )

            # ---- FFN1 (DR fp8): g^T = gelu(w1^T @ h2^T / 64 + b1) ----
            gT = [
                big.tile([P, 2, NQ], FP8, name=f"gT{s}", tag=f"gT{s}")
                for s in range(SM)
            ]
            for m in range(MC):
                ps = pp_pool.tile([P, E], F32, name="pp", tag="pp")
                for s in range(SE):
                    nc.tensor.matmul(
                        ps[:], w_slice(wt["w18"], s, m * P, (m + 1) * P),
                        h2T[s][:], start=(s == 0), stop=(s == SE - 1), perf_mode=DRM,
                    )
                nc.scalar.activation(
                    gT[m // 2][:, m % 2, :], ps[:], AF.Gelu,
                    bias=wt["b1"][:, m:m + 1], scale=1.0 / WS,
                )

            # ---- FFN2 (DR fp8) + b2 row + residual ----
            for qc in range(QC):
                ps = pp_pool.tile([P, E], F32, name="pp", tag="pp")
                for s in range(SM):
                    nc.tensor.matmul(
                        ps[:], gT[s][:, :, qc * P:(qc + 1) * P],
                        w_slice(wt["w28"], s, 0, E, SM),
                        start=(s == 0), stop=False, perf_mode=DRM,
                    )
                nc.tensor.matmul(
                    ps[:], ones1[:], wt["b2_row"][:], start=False, stop=True,
                    skip_group_check=True,
                )
                nc.vector.scalar_tensor_tensor(
                    xq[qc], ps[:], 1.0 / WS, xq[qc], op0=ALU.mult, op1=ALU.add,
                )

        for qc in range(QC):
            nc.sync.dma_start(y_d[qc * P:(qc + 1) * P, :], xq[qc])

    nc.compile()
    return nc


def get_nc():
    if "nc" not in _CACHE:
        _CACHE["nc"] = _build()
    return _CACHE["nc"]


def _fp8(a):
    return np.clip(np.asarray(a, np.float32), -240.0, 240.0).astype(
        ml_dtypes.float8_e4m3
    )


def _bf16(a):
    return np.asarray(a, np.float32).astype(ml_dtypes.bfloat16)


def _rearr_dr(w8, S):
    """[S*2*128, C] (row-major contraction) -> [128, S*2*C] DR pair layout."""
    C = w8.shape[1]
    return np.ascontiguousarray(
        w8.reshape(S, 2, P, C).transpose(2, 0, 1, 3).reshape(P, S * 2 * C)
    )


def _cols(v):
    """[k*128] -> [128, k]: column m holds v[m*128:(m+1)*128]."""
    k = v.shape[0] // P
    return np.ascontiguousarray(np.asarray(v, np.float32).reshape(k, P).T)


def kernel(**inputs) -> np.ndarray:
    x_q = np.asarray(inputs["x_q"], np.float32)
    x_kv = np.asarray(inputs["x_kv"], np.float32)
    wq = np.asarray(inputs["wq"], np.float32)
    wkv = np.asarray(inputs["wkv"], np.float32)
    wo = np.asarray(inputs["wo"], np.float32)
    bo = np.asarray(inputs["bo"], np.float32)
    w1 = np.asarray(inputs["w1"], np.float32)
    b1 = np.asarray(inputs["b1"], np.float32)
    w2 = np.asarray(inputs["w2"], np.float32)
    b2 = np.asarray(inputs["b2"], np.float32)
    ln1_g = np.asarray(inputs["ln1_g"], np.float32)
    ln1_b = np.asarray(inputs["ln1_b"], np.float32)
    ln2_g = np.asarray(inputs["ln2_g"], np.float32)
    ln2_b = np.asarray(inputs["ln2_b"], np.float32)

    shared = {}
    for l in range(L):
        wk_f = wkv[l][:, :E]
        wv_f = wkv[l][:, E:]
        wq_eff = ln1_g[l][:, None] * wq[l]
        wk_eff = ln1_g[l][:, None] * wk_f
        wv_eff = ln1_g[l][:, None] * wv_f
        bq_eff = ln1_b[l] @ wq[l]
        bv_eff = ln1_b[l] @ wv_f
        bo_eff = bo[l] + bv_eff @ wo[l]
        w1_eff = ln2_g[l][:, None] * w1[l]
        b1_eff = ln2_b[l] @ w1[l] + b1[l]
        shared.update({
            f"wq8_{l}": _rearr_dr(_fp8(WS * wq_eff), SE),
            f"wk8_{l}": _rearr_dr(_fp8(WS * wk_eff), SE),
            f"wv8_{l}": _rearr_dr(_fp8(WS * wv_eff), SE),
            f"wo8_{l}": _rearr_dr(_fp8(WS * wo[l]), SE),
            f"w18_{l}": _rearr_dr(_fp8(WS * w1_eff), SE),
            f"w28_{l}": _rearr_dr(_fp8(WS * w2[l]), SM),
            f"bq_{l}": _cols(QKS * bq_eff),
            f"b1_{l}": _cols(b1_eff),
            f"bo_row_{l}": _bf16(AOS * WS * bo_eff)[None, :],
            f"b2_row_{l}": _bf16(WS * b2[l])[None, :],
        })

    in_maps = []
    for c in range(8):
        b, qc = c // 4, c % 4
        m = dict(shared)
        m["xq"] = np.ascontiguousarray(x_q[b, qc * NQ:(qc + 1) * NQ, :])
        m["xkv"] = np.ascontiguousarray(x_kv[b])
        in_maps.append(m)

    nc = get_nc()
    res = bass_utils.run_bass_kernel_spmd(nc, in_maps, core_ids=list(range(8)))

    out = np.empty((2, 2048, E), np.float32)
    for c in range(8):
        b, qc = c // 4, c % 4
        out[b, qc * NQ:(qc + 1) * NQ, :] = res.results[c]["y"]
    return out
